# revision 18
# baseline (speedup 1.0000x reference)
"""Trainium2 Bass kernel for nn_CustomConvLayer (dynamic per-sample conv).

Sharding: pure data parallel over batch B=32 across 8 NeuronCores (4
samples per core). Small synthesis networks are replicated per core.

Per sample, on device:
  wm-embedding MLP -> per-channel modulation wm_coff  (tiny matmuls)
  t0 = sum-pool2(x)                                   (two DVE pair-adds)
  conv1 with w1s = w1 * (0.25*wm_coff)                (col-tiled 9-tap matmuls)
  conv2/conv3 (lrelu fused in ACT evacuation)         (col-tiled matmuls)
  4 coeff heads + attention head (conv+GAP)           (tap-stacked K=128 +
                                                       col-split M matmuls)
  w_dyn synthesis (4 experts)                         (DVE scalar_tensor_tensor)
  main conv: out = conv(x, w_dyn*wm_coff), 3x3 pad 1  (bf16 matmuls, 9 taps
                                                       accumulated in PSUM,
                                                       2 row-groups col-tiled)
The wm_coff modulation of x is folded into the conv1 weights (tower side)
and into the synthesized weights (main conv side), so the full-res image is
never rescaled and pooling needs no weights.
"""

import sys

if "/opt/trn_rl_repo" not in sys.path:
    sys.path.insert(0, "/opt/trn_rl_repo")

import numpy as np
import ml_dtypes
from contextlib import ExitStack

import concourse.bass as bass
import concourse.bacc as bacc
import concourse.tile as tile
from concourse import mybir
from concourse.bass_utils import run_bass_kernel_spmd

F32 = mybir.dt.float32
BF16 = mybir.dt.bfloat16
U32 = mybir.dt.uint32
AF = mybir.ActivationFunctionType
OP = mybir.AluOpType


class Cfg:
    def __init__(self, BL=4, Cin=128, H=128, W=128, n_cores=8):
        self.BL, self.Cin, self.H, self.W, self.n_cores = BL, Cin, H, W, n_cores
        self.Cout = 64
        # image padded to [H+2, W+4]: 1 row top/bottom, 2 cols left/right
        # (2-col pad keeps pooling reads 4B-aligned for the DVE 2x mode)
        self.Hp, self.Wp = H + 2, W + 4
        self.NPAD = self.Hp * self.Wp
        # pooled size and tower conv output sizes
        self.P, self.PW = H // 2, W // 2
        self.c1h, self.c1w = self.P - 2, self.PW - 2
        self.c2h, self.c2w = (self.c1h - 3) // 2 + 1, (self.c1w - 3) // 2 + 1
        self.c3h, self.c3w = (self.c2h - 3) // 2 + 1, (self.c2w - 3) // 2 + 1
        self.hh, self.hw = (self.c3h - 3) // 2 + 1, (self.c3w - 3) // 2 + 1
        self.gapn = self.hh * self.hw
        # main conv row groups: pairs of row-groups share one PSUM bank
        self.RPG = 512 // W          # rows per row-group (N = RPG*W = 512)
        assert H % (2 * self.RPG) == 0
        self.NRG = H // self.RPG
        self.NPAIR = self.NRG // 2
        self.POUT = min(4, self.NPAIR)   # psum-pairs per output staging tile
        assert self.NPAIR % self.POUT == 0
        self.NOUT = self.NPAIR // self.POUT
        # conv1 row blocks (paired into col groups)
        rb = max(1, min(self.c1h, 512 // self.c1w))
        self.blk1 = []
        y = 0
        while y < self.c1h:
            n = min(rb, self.c1h - y)
            self.blk1.append((y, n))
            y += n
        assert len(self.blk1) % 2 == 0
        # conv2 row blocks
        rb2 = max(1, min(self.c2h, 512 // self.c2w))
        self.blk2 = [(0, rb2), (rb2, self.c2h - rb2)]
        # conv3 row blocks
        h3a = self.c3h // 2
        self.blk3 = [(0, h3a), (h3a, self.c3h - h3a)]

        # ---- const blob layouts (uint32 columns) ----
        def layout(*sizes):
            offs, o = [], 0
            for n in sizes:
                offs.append(o)
                o += n
            return offs + [o]

        # bwm: wm-embedding + attention small weights (f32)
        (self.WM2, self.WM1, self.WMT, self.AFW, self.AFB, self.WB1, self.WB2,
         self.ONE, self.NWM) = layout(Cin, Cin, BL, 4, 4, 1, 1, Cin)
        # btw: tower conv weights (bf16) + biases (f32)
        (self.W1, self.W2, self.W3, self.TB, self.HB, self.NTW) = layout(
            9 * self.Cout // 2, 9 * self.Cout // 2, 9 * self.Cout // 2, 3, 5)
        # bhd: head weights bf16; per head: 3 stacked [128,128] + 3 single
        # [64,128] matrices, each 64 u32 cols
        self.NHD = 5 * 6 * 64
        # bex: expert table bf16 [Cin, 4*9*64]
        self.NEX = 4 * 9 * self.Cout // 2


def _pack_f32(dst, col, arr, row=0):
    a = np.ascontiguousarray(arr, dtype=np.float32)
    dst[row : row + a.shape[0], col : col + a.shape[1]] = a.view(np.uint32)


def _pack_bf16(dst, col, arr, row=0):
    a = np.ascontiguousarray(arr, dtype=ml_dtypes.bfloat16)
    u16 = a.view(np.uint16)
    u32 = (u16[:, 1::2].astype(np.uint32) << 16) | u16[:, 0::2].astype(np.uint32)
    dst[row : row + a.shape[0], col : col + u32.shape[1]] = u32


def make_blobs(cfg, wm_core, wm_w1, wm_b1, wm_w2, wm_b2, tr_w1, tr_b1, tr_w2,
               tr_b2, tr_w3, tr_b3, t1_w, t1_b, t2_w, t2_b, t3_w, t3_b, t4_w,
               t4_b, att_cw, att_cb, att_fw, att_fb, expert_w):
    bwm = np.zeros((128, cfg.NWM), np.uint32)
    _pack_f32(bwm, cfg.WM2, wm_w2.T)
    _pack_f32(bwm, cfg.WM1, wm_w1.T)
    _pack_f32(bwm, cfg.WMT, wm_core.T)
    _pack_f32(bwm, cfg.AFW, att_fw.T)
    _pack_f32(bwm, cfg.AFB, (att_fb / cfg.gapn)[None, :])
    _pack_f32(bwm, cfg.WB1, wm_b1[:, None])
    _pack_f32(bwm, cfg.WB2, wm_b2[:, None])
    _pack_f32(bwm, cfg.ONE, np.ones((1, cfg.Cin), np.float32))

    btw = np.zeros((128, cfg.NTW), np.uint32)
    _pack_bf16(btw, cfg.W1, tr_w1.transpose(1, 2, 3, 0).reshape(cfg.Cin, -1))
    _pack_bf16(btw, cfg.W2, tr_w2.transpose(1, 2, 3, 0).reshape(64, -1))
    _pack_bf16(btw, cfg.W3, tr_w3.transpose(1, 2, 3, 0).reshape(64, -1))
    _pack_f32(btw, cfg.TB, np.stack([tr_b1, tr_b2, tr_b3], 1))
    _pack_f32(btw, cfg.HB, np.stack([t1_b, t2_b, t3_b, t4_b, att_cb], 1))

    # heads: per head, per ky: stacked [(ky,0); (ky,1)] as [128,128], then
    # per ky: single (ky,2) as [64,128]
    bhd = np.zeros((128, cfg.NHD), np.uint32)
    col = 0
    for w in (t1_w, t2_w, t3_w, t4_w, att_cw):
        wt = w.transpose(1, 2, 3, 0)  # [i, ky, kx, o]
        for ky in range(3):
            _pack_bf16(bhd, col, np.concatenate([wt[:, ky, 0], wt[:, ky, 1]], 0))
            col += 64
        for ky in range(3):
            _pack_bf16(bhd, col, wt[:, ky, 2])
            col += 64

    bex = np.zeros((128, cfg.NEX), np.uint32)
    expT = expert_w[0].transpose(2, 0, 3, 4, 1).reshape(cfg.Cin, -1)
    _pack_bf16(bex, 0, expT)
    return bwm, btw, bhd, bex


def build_nc(cfg):
    nc = bacc.Bacc()
    Cin, Cout, H, W = cfg.Cin, cfg.Cout, cfg.H, cfg.W
    xin = nc.declare_dram_parameter("x", [cfg.BL, Cin, cfg.NPAD], BF16,
                                    isOutput=False)
    bwm_d = nc.declare_dram_parameter("bwm", [128, cfg.NWM], U32, isOutput=False)
    btw_d = nc.declare_dram_parameter("btw", [128, cfg.NTW], U32, isOutput=False)
    bhd_d = nc.declare_dram_parameter("bhd", [128, cfg.NHD], U32, isOutput=False)
    bex_d = nc.declare_dram_parameter("bex", [128, cfg.NEX], U32, isOutput=False)
    y = nc.declare_dram_parameter("y", [cfg.BL, cfg.NOUT, 128, cfg.POUT * 512],
                                  BF16, isOutput=True)

    with tile.TileContext(nc) as tc, ExitStack() as ctx:
        cpool = ctx.enter_context(tc.tile_pool(name="consts", bufs=1))
        xpool = ctx.enter_context(tc.tile_pool(name="xpad", bufs=1))
        dpool = ctx.enter_context(tc.tile_pool(name="data", bufs=1))
        spool = ctx.enter_context(tc.tile_pool(name="smalls", bufs=2))
        ypool = ctx.enter_context(tc.tile_pool(name="synth", bufs=2))
        wpool = ctx.enter_context(tc.tile_pool(name="wdyn", bufs=2))
        opool = ctx.enter_context(tc.tile_pool(name="outsb", bufs=2))
        mpsum = ctx.enter_context(tc.tile_pool(name="mpsum", bufs=3, space="PSUM"))
        tpsum = ctx.enter_context(tc.tile_pool(name="tpsum", bufs=3, space="PSUM"))
        hpsum = ctx.enter_context(tc.tile_pool(name="hpsum", bufs=2, space="PSUM"))

        bwm = cpool.tile([128, cfg.NWM], U32)
        btw = cpool.tile([128, cfg.NTW], U32)
        bhd = cpool.tile([128, cfg.NHD], U32)
        bex = cpool.tile([128, cfg.NEX], U32)
        nc.gpsimd.dma_start(bwm[:], bwm_d[:])
        nc.gpsimd.dma_start(btw[:], btw_d[:])

        def bl(t, c0, c1, nrows=128, dt=F32, row=0):
            return t[row : row + nrows, c0:c1].bitcast(dt)

        wm_w2T = bl(bwm, cfg.WM2, cfg.WM1)
        wm_w1T = bl(bwm, cfg.WM1, cfg.WMT, 32)
        wmT = bl(bwm, cfg.WMT, cfg.AFW, 32)
        att_fwT = bl(bwm, cfg.AFW, cfg.AFB)
        att_fb = bl(bwm, cfg.AFB, cfg.WB1, 1)
        wm_b1 = bl(bwm, cfg.WB1, cfg.WB2)
        wm_b2 = bl(bwm, cfg.WB2, cfg.ONE)
        ones_row = bl(bwm, cfg.ONE, cfg.NWM, 1)
        w1T = bl(btw, cfg.W1, cfg.W2, 128, BF16)
        w2T = bl(btw, cfg.W2, cfg.W3, 64, BF16)
        w3T = bl(btw, cfg.W3, cfg.TB, 64, BF16)
        tr_b = bl(btw, cfg.TB, cfg.HB, 64)
        head_b = bl(btw, cfg.HB, cfg.NTW)
        expT = bex[:].bitcast(BF16)

        def hd_stk(h, ky, g):
            ap = bl(bhd, (h * 6 + ky) * 64, (h * 6 + ky + 1) * 64, 128, BF16)
            return ap[:, 64 * g : 64 * g + 64]

        def hd_sgl(h, ky, g):
            ap = bl(bhd, (h * 6 + 3 + ky) * 64, (h * 6 + 4 + ky) * 64, 64, BF16)
            return ap[:, 64 * g : 64 * g + 64]

        # wm-embedding scratch (written once, read per-sample)
        wmx = cpool.tile([128, 3 * cfg.BL], F32)
        hT = wmx[:, 0 : cfg.BL]
        wmc = wmx[:, cfg.BL : 2 * cfg.BL]
        wq = wmx[:, 2 * cfg.BL : 3 * cfg.BL]

        nxp = 3
        xpads = [
            xpool.tile([128, cfg.NPAD], BF16, tag=f"xp{i}", name=f"xp{i}")
            for i in range(nxp)
        ]
        xvs = [xp[:].rearrange("p (r c) -> p r c", c=cfg.Wp) for xp in xpads]

        scr = dpool.tile([128, cfg.PW * 32], BF16, tag="scr")
        n_t1, n_t2 = cfg.c1h * cfg.c1w, cfg.c2h * cfg.c2w
        tow = dpool.tile([64, n_t1 + n_t2], BF16, tag="tower")
        t1v = tow[:, 0:n_t1].rearrange("p (r c) -> p r c", c=cfg.c1w)
        t2v = tow[:, n_t1:].rearrange("p (r c) -> p r c", c=cfg.c2w)
        t3t = dpool.tile([128, cfg.c3h * cfg.c3w], BF16, tag="t3")
        t3v = t3t[:].rearrange("p (r c) -> p r c", c=cfg.c3w)

        t0s, wdyns, w1ss = {}, {}, {}

        # ---- wm embedding -> wm_coff.T [Cin, BL] (once, all samples) ----
        ps = hpsum.tile([128, cfg.BL], F32, tag="hps")
        nc.tensor.matmul(ps[:], wm_w1T, wmT, start=True, stop=True)
        nc.scalar.activation(hT, ps[:], AF.Prelu, bias=wm_b1, alpha=0.2)
        ps = hpsum.tile([128, cfg.BL], F32, tag="hps")
        nc.tensor.matmul(ps[:], wm_w2T, hT, start=True, stop=True)
        nc.scalar.activation(wmc, ps[:], AF.Identity, bias=wm_b2)
        nc.scalar.activation(wq, wmc, AF.Copy, scale=0.25)

        # ---------- per-sample stage emitters ----------
        def emit_dma(s, nchunks=4):
            xp = xpads[s % nxp]
            step = (cfg.Hp + nchunks - 1) // nchunks * cfg.Wp
            for c0 in range(0, cfg.NPAD, step):
                c1 = min(cfg.NPAD, c0 + step)
                nc.gpsimd.dma_start(xp[:, c0:c1], xin[s, :, c0:c1])

        def emit_w1s(s):
            # on ScalarE so the DVE queue stays clear for pooling
            w1s = dpool.tile([128, 9 * 64], BF16, tag="w1s", bufs=2,
                             name=f"w1s_{s}")
            w1ss[s] = w1s
            nc.scalar.activation(w1s[:], w1T, AF.Copy, scale=wq[:, s : s + 1])

        def emit_pool(s, nchunks=4):
            # 2x2 sum-pool: row-pair add (bf16 2x) then col-pair add
            t0 = dpool.tile([128, cfg.P * cfg.PW], BF16, tag="t0", bufs=2,
                            name=f"t0_{s}")
            t0s[s] = t0
            xv = xvs[s % nxp]
            for q in range(nchunks):
                r0 = (cfg.P // nchunks) * q    # pooled row of chunk start
                nr = cfg.P // nchunks          # pooled rows per chunk
                sc = scr[:, 0 : nr * cfg.W].rearrange("p (r c) -> p r c", c=cfg.W)
                nc.vector.tensor_add(
                    sc,
                    xv[:, 1 + 2 * r0 : 1 + 2 * (r0 + nr) : 2, 2 : 2 + cfg.W],
                    xv[:, 2 + 2 * r0 : 2 + 2 * (r0 + nr) : 2, 2 : 2 + cfg.W],
                )
                t0c = t0[:, r0 * cfg.PW : (r0 + nr) * cfg.PW].rearrange(
                    "p (r c) -> p r c", c=cfg.PW
                )
                nc.vector.tensor_add(
                    t0c, sc[:, :, 0 : cfg.W : 2], sc[:, :, 1 : cfg.W : 2]
                )

        def emit_conv1(s):
            t0v = t0s.pop(s)[:].rearrange("p (r c) -> p r c", c=cfg.PW)
            w1s = w1ss.pop(s)[:]
            for p in range(len(cfg.blk1) // 2):
                (yA, nA), (yB, nB) = cfg.blk1[2 * p], cfg.blk1[2 * p + 1]
                ps = tpsum.tile([128, nA * cfg.c1w], F32, tag="tps")
                for ky in range(3):
                    for kx in range(3):
                        t = ky * 3 + kx
                        lhs = w1s[:, t * 64 : (t + 1) * 64]
                        st, sp = t == 0, t == 8
                        nc.tensor.matmul(
                            ps[0:64, 0 : nA * cfg.c1w], lhs,
                            t0v[:, yA + ky : yA + ky + nA, kx : kx + cfg.c1w],
                            start=st, stop=sp,
                        )
                        nc.tensor.matmul(
                            ps[64:128, 0 : nB * cfg.c1w], lhs,
                            t0v[:, yB + ky : yB + ky + nB, kx : kx + cfg.c1w],
                            start=st, stop=sp,
                        )
                nc.scalar.activation(t1v[:, yA : yA + nA, :],
                                     ps[0:64, 0 : nA * cfg.c1w],
                                     AF.Lrelu, bias=tr_b[:, 0:1], alpha=0.01)
                nc.scalar.activation(t1v[:, yB : yB + nB, :],
                                     ps[64:128, 0 : nB * cfg.c1w],
                                     AF.Lrelu, bias=tr_b[:, 0:1], alpha=0.01)

        def emit_conv23(s):
            # conv2: two row blocks col-tiled
            (yA, nA), (yB, nB) = cfg.blk2
            ps = tpsum.tile([128, nA * cfg.c2w], F32, tag="tps")
            for ky in range(3):
                for kx in range(3):
                    t = ky * 3 + kx
                    lhs = w2T[:, t * 64 : (t + 1) * 64]
                    st, sp = t == 0, t == 8
                    nc.tensor.matmul(
                        ps[0:64, 0 : nA * cfg.c2w], lhs,
                        t1v[:, 2 * yA + ky : 2 * yA + ky + 2 * nA : 2,
                            kx : kx + 2 * cfg.c2w - 1 : 2],
                        start=st, stop=sp,
                    )
                    nc.tensor.matmul(
                        ps[64:128, 0 : nB * cfg.c2w], lhs,
                        t1v[:, 2 * yB + ky : 2 * yB + ky + 2 * nB : 2,
                            kx : kx + 2 * cfg.c2w - 1 : 2],
                        start=st, stop=sp,
                    )
            nc.scalar.activation(t2v[:, yA : yA + nA, :],
                                 ps[0:64, 0 : nA * cfg.c2w],
                                 AF.Lrelu, bias=tr_b[:, 1:2], alpha=0.01)
            nc.scalar.activation(t2v[:, yB : yB + nB, :],
                                 ps[64:128, 0 : nB * cfg.c2w],
                                 AF.Lrelu, bias=tr_b[:, 1:2], alpha=0.01)

            # conv3: two row blocks col-tiled
            (yA, nA), (yB, nB) = cfg.blk3
            ps = tpsum.tile([128, nA * cfg.c3w], F32, tag="tps")
            for ky in range(3):
                for kx in range(3):
                    t = ky * 3 + kx
                    lhs = w3T[:, t * 64 : (t + 1) * 64]
                    st, sp = t == 0, t == 8
                    nc.tensor.matmul(
                        ps[0:64, 0 : nA * cfg.c3w], lhs,
                        t2v[:, 2 * yA + ky : 2 * yA + ky + 2 * nA : 2,
                            kx : kx + 2 * cfg.c3w - 1 : 2],
                        start=st, stop=sp,
                    )
                    nc.tensor.matmul(
                        ps[64:128, 0 : nB * cfg.c3w], lhs,
                        t2v[:, 2 * yB + ky : 2 * yB + ky + 2 * nB : 2,
                            kx : kx + 2 * cfg.c3w - 1 : 2],
                        start=st, stop=sp,
                    )
            nc.scalar.activation(t3v[0:64, yA : yA + nA, :],
                                 ps[0:64, 0 : nA * cfg.c3w],
                                 AF.Lrelu, bias=tr_b[:, 2:3], alpha=0.01)
            nc.scalar.activation(t3v[0:64, yB : yB + nB, :],
                                 ps[64:128, 0 : nB * cfg.c3w],
                                 AF.Lrelu, bias=tr_b[:, 2:3], alpha=0.01)
            # duplicate t3 shifted by one col onto partitions 64-127
            # (tap-stacking source for the heads)
            nc.vector.tensor_copy(
                t3v[64:128, :, 0 : cfg.c3w - 1], t3v[0:64, :, 1 : cfg.c3w]
            )

        def emit_heads_att_synth(s):
            sm = spool.tile([128, 64], F32, tag="sm", name=f"sm_{s}")
            a_sb = sm[:, 0:1]
            att_row = sm[0:1, 4:8]
            att_bc = sm[:, 8:12]
            cc = sm[:, 12:16]
            gap = sm[:, 16:24]
            hscr = sm[:, 24:42].bitcast(BF16)[:, 0 : cfg.gapn]
            nh, nw = cfg.hh, cfg.hw
            for h in range(5):
                ps = hpsum.tile([128, cfg.gapn], F32, tag="hps")
                for ky in range(3):
                    for g in range(2):
                        nc.tensor.matmul(
                            ps[64 * g : 64 * g + 64, :], hd_stk(h, ky, g),
                            t3v[:, ky : ky + 2 * nh - 1 : 2, 0 : 2 * nw - 1 : 2],
                            start=(ky == 0), stop=False,
                        )
                for ky in range(3):
                    for g in range(2):
                        nc.tensor.matmul(
                            ps[64 * g : 64 * g + 64, :], hd_sgl(h, ky, g),
                            t3v[0:64, ky : ky + 2 * nh - 1 : 2,
                                2 : 2 + 2 * nw - 1 : 2],
                            start=False, stop=(ky == 2),
                        )
                nc.scalar.activation(
                    hscr, ps[:], AF.Identity, bias=head_b[:, h : h + 1],
                    accum_out=gap[:, h : h + 1],
                )

            # attention: a = lrelu(gap4/gapn); att = (a@att_fwT + fb)/gapn
            nc.scalar.activation(a_sb, gap[:, 4:5], AF.Lrelu,
                                 scale=1.0 / cfg.gapn, alpha=0.01)
            ps = hpsum.tile([1, 4], F32, tag="hps")
            nc.tensor.matmul(ps[:], a_sb, att_fwT, start=True, stop=True)
            nc.vector.scalar_tensor_tensor(
                att_row, ps[:], 1.0 / cfg.gapn, att_fb, op0=OP.mult, op1=OP.add
            )
            ps = hpsum.tile([128, 4], F32, tag="hps")
            nc.tensor.matmul(ps[:], ones_row, att_row, start=True, stop=True)
            nc.scalar.activation(att_bc, ps[:], AF.Copy)
            nc.vector.tensor_mul(cc, att_bc, gap[:, 0:4])

            # synthesize w_dynT[i, (kh kw o)], fold in wm_coff
            A = ypool.tile([128, 9 * 64], BF16, tag="synA", name=f"synA_{s}")
            Bt = ypool.tile([128, 9 * 64], BF16, tag="synB", name=f"synB_{s}")
            wdyn = wpool.tile([128, 9 * 64], BF16, tag="wdyn", name=f"wdyn_{s}")
            wdyns[s] = wdyn
            nc.vector.tensor_scalar_mul(A[:], expT[:, 0:576], cc[:, 0:1])
            nc.vector.scalar_tensor_tensor(
                Bt[:], expT[:, 576:1152], cc[:, 1:2], A[:], op0=OP.mult,
                op1=OP.add,
            )
            nc.vector.scalar_tensor_tensor(
                A[:], expT[:, 1152:1728], cc[:, 2:3], Bt[:], op0=OP.mult,
                op1=OP.add,
            )
            nc.vector.scalar_tensor_tensor(
                Bt[:], expT[:, 1728:2304], cc[:, 3:4], A[:], op0=OP.mult,
                op1=OP.add,
            )
            nc.vector.tensor_scalar_mul(wdyn[:], Bt[:], wmc[:, s : s + 1])

        def emit_main_group(s, q):
            xv = xvs[s % nxp]
            wdyn = wdyns[s]
            out_t = opool.tile([128, cfg.POUT * 512], BF16, tag="outsb",
                               name=f"out_{s}_{q}")
            for j in range(cfg.POUT):
                pair = q * cfg.POUT + j
                # even/odd row-groups stream concurrently into the two
                # PE column groups
                ps = mpsum.tile([128, 512], F32, tag="mps")
                for ky in range(3):
                    for kx in range(3):
                        for half in range(2):
                            y0 = (2 * pair + half) * cfg.RPG
                            nc.tensor.matmul(
                                ps[half * 64 : half * 64 + 64, :],
                                wdyn[:, (ky * 3 + kx) * 64 : (ky * 3 + kx + 1) * 64],
                                xv[:, y0 + ky : y0 + ky + cfg.RPG,
                                   1 + kx : 1 + kx + cfg.W],
                                start=(ky == 0 and kx == 0),
                                stop=(ky == 2 and kx == 2),
                            )
                nc.scalar.activation(
                    out_t[:, j * 512 : (j + 1) * 512], ps[:], AF.Copy
                )
                # last sample: stream the output out per psum-pair so the
                # final DMA tail is short
                if s == cfg.BL - 1 and j % 2 == 1:
                    nc.gpsimd.dma_start(
                        y[s, q, :, (j - 1) * 512 : (j + 1) * 512],
                        out_t[:, (j - 1) * 512 : (j + 1) * 512],
                    )
            if s != cfg.BL - 1:
                nc.gpsimd.dma_start(y[s, q], out_t[:])
            if q == cfg.NOUT - 1:
                wdyns.pop(s)

        # ---------- software pipeline ----------
        emit_dma(0, nchunks=8)
        nc.gpsimd.dma_start(bhd[:], bhd_d[:])
        nc.gpsimd.dma_start(bex[:], bex_d[:])
        # PE warm-up: dummy matmuls on the const blob while the image DMA
        # lands, so the HAM clock gate is released before conv1 starts
        wps = mpsum.tile([128, 512], F32, tag="mps", name="warmup")
        wrhs = bwm[:].bitcast(BF16)
        for i in range(40):
            nc.tensor.matmul(wps[0:64, 0:512], wrhs[:, 0:64],
                             wrhs[:, 256 : 256 + 512],
                             start=(i == 0), stop=(i == 39))
        emit_w1s(0)
        emit_pool(0, nchunks=8)
        if cfg.BL > 1:
            emit_dma(1)
        emit_conv1(0)
        emit_conv23(0)
        emit_heads_att_synth(0)
        if cfg.BL > 1:
            emit_w1s(1)
            emit_pool(1)

        def stage_after(s, q):
            if q == min(0, cfg.NOUT - 1):
                if s + 2 < cfg.BL:
                    emit_dma(s + 2)
                if s + 1 < cfg.BL:
                    emit_conv1(s + 1)
            if q == min(1, cfg.NOUT - 1):
                if s + 1 < cfg.BL:
                    emit_conv23(s + 1)
                    emit_heads_att_synth(s + 1)
            if q == min(2, cfg.NOUT - 1):
                if s + 2 < cfg.BL:
                    emit_w1s(s + 2)
                    emit_pool(s + 2)

        for s in range(cfg.BL):
            for q in range(cfg.NOUT):
                emit_main_group(s, q)
                stage_after(s, q)

    return nc


_NC_CACHE = {}
TRACE = False       # set by test harness to collect an NTFF profile
TRACE_DIR = None    # where to leave the NTFF/perfetto artifacts
LAST_RESULT = None  # BassKernelResults of the most recent kernel() call


def _get_nc(cfg):
    key = (cfg.BL, cfg.Cin, cfg.H, cfg.W)
    if key not in _NC_CACHE:
        nc = build_nc(cfg)
        if not nc.is_finalized():
            nc.finalize()
        _NC_CACHE[key] = nc
    return _NC_CACHE[key]


def pad_images(cfg, x):
    """[n, Cin, H, W] -> zero-padded flat bf16 [n, Cin, Hp*Wp]."""
    n = x.shape[0]
    xp = np.zeros((n, cfg.Cin, cfg.Hp, cfg.Wp), ml_dtypes.bfloat16)
    xp[:, :, 1 : cfg.H + 1, 2 : cfg.W + 2] = x.astype(ml_dtypes.bfloat16)
    return xp.reshape(n, cfg.Cin, cfg.NPAD)


def unpack_y(cfg, yraw):
    """[BL, NOUT, 128, POUT*512] bf16 -> [BL, 64, H, W] f32."""
    a = np.asarray(yraw).astype(np.float32)
    a = a.reshape(cfg.BL, cfg.NOUT, 2, 64, cfg.POUT, cfg.RPG, cfg.W)
    a = a.transpose(0, 3, 1, 4, 2, 5, 6)
    return np.ascontiguousarray(a.reshape(cfg.BL, 64, cfg.H, cfg.W))


def kernel(**inputs):
    x = np.asarray(inputs["x"], np.float32)
    B, Cin, H, W = x.shape
    cfg = Cfg(BL=B // 8, Cin=Cin, H=H, W=W)
    nc = _get_nc(cfg)
    wnames = [
        "wm_w1", "wm_b1", "wm_w2", "wm_b2", "tr_w1", "tr_b1", "tr_w2", "tr_b2",
        "tr_w3", "tr_b3", "t1_w", "t1_b", "t2_w", "t2_b", "t3_w", "t3_b",
        "t4_w", "t4_b", "att_cw", "att_cb", "att_fw", "att_fb", "expert_w",
    ]
    ws = {k: np.asarray(inputs[k], np.float32) for k in wnames}
    wm = np.asarray(inputs["wm"], np.float32)
    in_maps = []
    for c in range(8):
        sl = slice(c * cfg.BL, (c + 1) * cfg.BL)
        bwm, btw, bhd, bex = make_blobs(cfg, wm[sl], **ws)
        in_maps.append({"x": pad_images(cfg, x[sl]), "bwm": bwm, "btw": btw,
                        "bhd": bhd, "bex": bex})
    global LAST_RESULT
    kw = {"tmpdir": TRACE_DIR} if (TRACE and TRACE_DIR) else {}
    res = run_bass_kernel_spmd(nc, in_maps, list(range(8)), trace=TRACE, **kw)
    LAST_RESULT = res
    return np.concatenate(
        [unpack_y(cfg, res.results[c]["y"]) for c in range(8)], axis=0
    )


# revision 27
# speedup vs baseline: 1.0145x; 1.0145x over previous
"""Trainium2 Bass kernel for nn_CustomConvLayer (dynamic per-sample conv).

Sharding: pure data parallel over batch B=32 across 8 NeuronCores (4
samples per core). Small synthesis networks are replicated per core.

Per sample, on device:
  wm-embedding MLP -> per-channel modulation wm_coff  (tiny matmuls)
  t0 = sum-pool2(x)                                   (two DVE pair-adds)
  conv1 with w1s = w1 * (0.25*wm_coff)                (col-tiled 9-tap matmuls)
  conv2/conv3 (lrelu fused in ACT evacuation)         (col-tiled matmuls)
  4 coeff heads + attention head (conv+GAP)           (tap-stacked K=128 +
                                                       col-split M matmuls)
  w_dyn synthesis (4 experts)                         (DVE scalar_tensor_tensor)
  main conv: out = conv(x, w_dyn*wm_coff), 3x3 pad 1  (bf16 matmuls, 9 taps
                                                       accumulated in PSUM,
                                                       2 row-groups col-tiled)
The wm_coff modulation of x is folded into the conv1 weights (tower side)
and into the synthesized weights (main conv side), so the full-res image is
never rescaled and pooling needs no weights.
"""

import sys

if "/opt/trn_rl_repo" not in sys.path:
    sys.path.insert(0, "/opt/trn_rl_repo")

import numpy as np
import ml_dtypes
from contextlib import ExitStack

import concourse.bass as bass
import concourse.bacc as bacc
import concourse.tile as tile
from concourse import mybir
from concourse.bass_utils import run_bass_kernel_spmd

F32 = mybir.dt.float32
BF16 = mybir.dt.bfloat16
U32 = mybir.dt.uint32
AF = mybir.ActivationFunctionType
OP = mybir.AluOpType


class Cfg:
    def __init__(self, BL=4, Cin=128, H=128, W=128, n_cores=8):
        self.BL, self.Cin, self.H, self.W, self.n_cores = BL, Cin, H, W, n_cores
        self.Cout = 64
        # image padded to [H+2, W+4]: 1 row top/bottom, 2 cols left/right
        # (2-col pad keeps pooling reads 4B-aligned for the DVE 2x mode)
        self.Hp, self.Wp = H + 2, W + 4
        self.NPAD = self.Hp * self.Wp
        # pooled size and tower conv output sizes
        self.P, self.PW = H // 2, W // 2
        self.c1h, self.c1w = self.P - 2, self.PW - 2
        self.c2h, self.c2w = (self.c1h - 3) // 2 + 1, (self.c1w - 3) // 2 + 1
        self.c3h, self.c3w = (self.c2h - 3) // 2 + 1, (self.c2w - 3) // 2 + 1
        self.hh, self.hw = (self.c3h - 3) // 2 + 1, (self.c3w - 3) // 2 + 1
        self.gapn = self.hh * self.hw
        # main conv row groups: pairs of row-groups share one PSUM bank
        self.RPG = 512 // W          # rows per row-group (N = RPG*W = 512)
        assert H % (2 * self.RPG) == 0
        self.NRG = H // self.RPG
        self.NPAIR = self.NRG // 2
        self.POUT = min(4, self.NPAIR)   # psum-pairs per output staging tile
        assert self.NPAIR % self.POUT == 0
        self.NOUT = self.NPAIR // self.POUT
        # conv1 row blocks (paired into col groups)
        rb = max(1, min(self.c1h, 512 // self.c1w))
        self.blk1 = []
        y = 0
        while y < self.c1h:
            n = min(rb, self.c1h - y)
            self.blk1.append((y, n))
            y += n
        assert len(self.blk1) % 2 == 0
        # conv2 row blocks
        rb2 = max(1, min(self.c2h, 512 // self.c2w))
        self.blk2 = [(0, rb2), (rb2, self.c2h - rb2)]
        # conv3 row blocks
        h3a = self.c3h // 2
        self.blk3 = [(0, h3a), (h3a, self.c3h - h3a)]

        # ---- const blob layouts (uint32 columns) ----
        def layout(*sizes):
            offs, o = [], 0
            for n in sizes:
                offs.append(o)
                o += n
            return offs + [o]

        # bwm: wm-embedding + attention small weights (f32)
        # ATT2: [2, 4] region — row 0 device-written att scratch, row 1 fb/gapn
        # ONE: [2, Cin] of ones (2 rows for the fb-folding broadcast matmul)
        (self.WM2, self.WM1, self.WMT, self.AFW, self.AT2, self.WB1, self.WB2,
         self.ONE, self.NWM) = layout(Cin, Cin, BL, 4, 4, 1, 1, Cin)
        # btw: tower conv weights (bf16) + biases (f32)
        (self.W1, self.W2, self.W3, self.TB, self.HB, self.NTW) = layout(
            9 * self.Cout // 2, 9 * self.Cout // 2, 9 * self.Cout // 2, 3, 5)
        # bhd: head weights bf16; per head: 3 stacked [128,128] + 3 single
        # [64,128] matrices, each 64 u32 cols
        self.NHD = 5 * 6 * 64
        # bex: expert table bf16 [Cin, 4*9*64]
        self.NEX = 4 * 9 * self.Cout // 2


def _pack_f32(dst, col, arr, row=0):
    a = np.ascontiguousarray(arr, dtype=np.float32)
    dst[row : row + a.shape[0], col : col + a.shape[1]] = a.view(np.uint32)


def _pack_bf16(dst, col, arr, row=0):
    a = np.ascontiguousarray(arr, dtype=ml_dtypes.bfloat16)
    u16 = a.view(np.uint16)
    u32 = (u16[:, 1::2].astype(np.uint32) << 16) | u16[:, 0::2].astype(np.uint32)
    dst[row : row + a.shape[0], col : col + u32.shape[1]] = u32


def make_blobs(cfg, wm_core, wm_w1, wm_b1, wm_w2, wm_b2, tr_w1, tr_b1, tr_w2,
               tr_b2, tr_w3, tr_b3, t1_w, t1_b, t2_w, t2_b, t3_w, t3_b, t4_w,
               t4_b, att_cw, att_cb, att_fw, att_fb, expert_w):
    bwm = np.zeros((128, cfg.NWM), np.uint32)
    _pack_f32(bwm, cfg.WM2, wm_w2.T)
    _pack_f32(bwm, cfg.WM1, wm_w1.T)
    _pack_f32(bwm, cfg.WMT, wm_core.T)
    _pack_f32(bwm, cfg.AFW, att_fw.T / cfg.gapn)
    _pack_f32(bwm, cfg.AT2, (att_fb / cfg.gapn)[None, :], row=1)
    _pack_f32(bwm, cfg.WB1, wm_b1[:, None])
    _pack_f32(bwm, cfg.WB2, wm_b2[:, None])
    _pack_f32(bwm, cfg.ONE, np.ones((2, cfg.Cin), np.float32))

    btw = np.zeros((128, cfg.NTW), np.uint32)
    _pack_bf16(btw, cfg.W1, tr_w1.transpose(1, 2, 3, 0).reshape(cfg.Cin, -1))
    _pack_bf16(btw, cfg.W2, tr_w2.transpose(1, 2, 3, 0).reshape(64, -1))
    _pack_bf16(btw, cfg.W3, tr_w3.transpose(1, 2, 3, 0).reshape(64, -1))
    _pack_f32(btw, cfg.TB, np.stack([tr_b1, tr_b2, tr_b3], 1))
    _pack_f32(btw, cfg.HB, np.stack([att_cb, t1_b, t2_b, t3_b, t4_b], 1))

    # heads (att head first): per head, per ky: stacked [(ky,0); (ky,1)] as
    # [128,128], then per ky: single (ky,2) as [64,128]
    bhd = np.zeros((128, cfg.NHD), np.uint32)
    col = 0
    for w in (att_cw, t1_w, t2_w, t3_w, t4_w):
        wt = w.transpose(1, 2, 3, 0)  # [i, ky, kx, o]
        for ky in range(3):
            _pack_bf16(bhd, col, np.concatenate([wt[:, ky, 0], wt[:, ky, 1]], 0))
            col += 64
        for ky in range(3):
            _pack_bf16(bhd, col, wt[:, ky, 2])
            col += 64

    bex = np.zeros((128, cfg.NEX), np.uint32)
    expT = expert_w[0].transpose(2, 0, 3, 4, 1).reshape(cfg.Cin, -1)
    _pack_bf16(bex, 0, expT)
    return bwm, btw, bhd, bex


def build_nc(cfg):
    nc = bacc.Bacc()
    Cin, Cout, H, W = cfg.Cin, cfg.Cout, cfg.H, cfg.W
    xin = nc.declare_dram_parameter("x", [cfg.BL, Cin, cfg.NPAD], BF16,
                                    isOutput=False)
    bwm_d = nc.declare_dram_parameter("bwm", [128, cfg.NWM], U32, isOutput=False)
    btw_d = nc.declare_dram_parameter("btw", [128, cfg.NTW], U32, isOutput=False)
    bhd_d = nc.declare_dram_parameter("bhd", [128, cfg.NHD], U32, isOutput=False)
    bex_d = nc.declare_dram_parameter("bex", [128, cfg.NEX], U32, isOutput=False)
    y = nc.declare_dram_parameter("y", [cfg.BL, cfg.NOUT, 128, cfg.POUT * 512],
                                  BF16, isOutput=True)

    with tile.TileContext(nc) as tc, ExitStack() as ctx:
        cpool = ctx.enter_context(tc.tile_pool(name="consts", bufs=1))
        xpool = ctx.enter_context(tc.tile_pool(name="xpad", bufs=1))
        dpool = ctx.enter_context(tc.tile_pool(name="data", bufs=1))
        spool = ctx.enter_context(tc.tile_pool(name="smalls", bufs=2))
        ypool = ctx.enter_context(tc.tile_pool(name="synth", bufs=2))
        wpool = ctx.enter_context(tc.tile_pool(name="wdyn", bufs=2))
        opool = ctx.enter_context(tc.tile_pool(name="outsb", bufs=2))
        mpsum = ctx.enter_context(tc.tile_pool(name="mpsum", bufs=3, space="PSUM"))
        tpsum = ctx.enter_context(tc.tile_pool(name="tpsum", bufs=3, space="PSUM"))
        hpsum = ctx.enter_context(tc.tile_pool(name="hpsum", bufs=2, space="PSUM"))

        bwm = cpool.tile([128, cfg.NWM], U32)
        btw = cpool.tile([128, cfg.NTW], U32)
        bhd = cpool.tile([128, cfg.NHD], U32)
        bex = cpool.tile([128, cfg.NEX], U32)
        nc.gpsimd.dma_start(bwm[:], bwm_d[:])
        nc.gpsimd.dma_start(btw[:], btw_d[:])

        def bl(t, c0, c1, nrows=128, dt=F32, row=0):
            return t[row : row + nrows, c0:c1].bitcast(dt)

        wm_w2T = bl(bwm, cfg.WM2, cfg.WM1)
        wm_w1T = bl(bwm, cfg.WM1, cfg.WMT, 32)
        wmT = bl(bwm, cfg.WMT, cfg.AFW, 32)
        att_fwT = bl(bwm, cfg.AFW, cfg.AT2)
        att2 = bl(bwm, cfg.AT2, cfg.WB1, 2)   # row0 scratch, row1 fb/gapn
        wm_b1 = bl(bwm, cfg.WB1, cfg.WB2)
        wm_b2 = bl(bwm, cfg.WB2, cfg.ONE)
        ones2 = bl(bwm, cfg.ONE, cfg.NWM, 2)
        w1T = bl(btw, cfg.W1, cfg.W2, 128, BF16)
        w2T = bl(btw, cfg.W2, cfg.W3, 64, BF16)
        w3T = bl(btw, cfg.W3, cfg.TB, 64, BF16)
        tr_b = bl(btw, cfg.TB, cfg.HB, 64)
        head_b = bl(btw, cfg.HB, cfg.NTW)
        expT = bex[:].bitcast(BF16)

        def hd_stk(h, ky, g):
            ap = bl(bhd, (h * 6 + ky) * 64, (h * 6 + ky + 1) * 64, 128, BF16)
            return ap[:, 64 * g : 64 * g + 64]

        def hd_sgl(h, ky, g):
            ap = bl(bhd, (h * 6 + 3 + ky) * 64, (h * 6 + 4 + ky) * 64, 64, BF16)
            return ap[:, 64 * g : 64 * g + 64]

        # wm-embedding scratch (written once, read per-sample)
        wmx = cpool.tile([128, 3 * cfg.BL], F32)
        hT = wmx[:, 0 : cfg.BL]
        wmc = wmx[:, cfg.BL : 2 * cfg.BL]
        wq = wmx[:, 2 * cfg.BL : 3 * cfg.BL]

        nxp = 3
        xpads = [
            xpool.tile([128, cfg.NPAD], BF16, tag=f"xp{i}", name=f"xp{i}")
            for i in range(nxp)
        ]
        xvs = [xp[:].rearrange("p (r c) -> p r c", c=cfg.Wp) for xp in xpads]

        scr = dpool.tile([128, cfg.PW * 32], BF16, tag="scr")
        n_t1, n_t2 = cfg.c1h * cfg.c1w, cfg.c2h * cfg.c2w
        tow = dpool.tile([64, n_t1 + n_t2], BF16, tag="tower")
        t1v = tow[:, 0:n_t1].rearrange("p (r c) -> p r c", c=cfg.c1w)
        t2v = tow[:, n_t1:].rearrange("p (r c) -> p r c", c=cfg.c2w)
        t3t = dpool.tile([128, cfg.c3h * cfg.c3w], BF16, tag="t3")
        t3v = t3t[:].rearrange("p (r c) -> p r c", c=cfg.c3w)

        t0s, wdyns, w1ss = {}, {}, {}

        # ---- wm embedding -> wm_coff.T [Cin, BL] (once, all samples) ----
        ps = hpsum.tile([128, cfg.BL], F32, tag="hps")
        nc.tensor.matmul(ps[:], wm_w1T, wmT, start=True, stop=True)
        nc.scalar.activation(hT, ps[:], AF.Prelu, bias=wm_b1, alpha=0.2)
        ps = hpsum.tile([128, cfg.BL], F32, tag="hps")
        nc.tensor.matmul(ps[:], wm_w2T, hT, start=True, stop=True)
        nc.scalar.activation(wmc, ps[:], AF.Identity, bias=wm_b2)
        nc.scalar.activation(wq, wmc, AF.Copy, scale=0.25)

        # ---------- per-sample stage emitters ----------
        def emit_dma(s, nchunks=4):
            xp = xpads[s % nxp]
            step = (cfg.Hp + nchunks - 1) // nchunks * cfg.Wp
            for c0 in range(0, cfg.NPAD, step):
                c1 = min(cfg.NPAD, c0 + step)
                nc.gpsimd.dma_start(xp[:, c0:c1], xin[s, :, c0:c1])

        def emit_w1s(s):
            # on ScalarE so the DVE queue stays clear for pooling
            w1s = dpool.tile([128, 9 * 64], BF16, tag="w1s", bufs=2,
                             name=f"w1s_{s}")
            w1ss[s] = w1s
            nc.scalar.activation(w1s[:], w1T, AF.Copy, scale=wq[:, s : s + 1])

        def emit_pool(s, nchunks=4):
            # 2x2 sum-pool: row-pair add (bf16 2x) then col-pair add
            t0 = dpool.tile([128, cfg.P * cfg.PW], BF16, tag="t0", bufs=2,
                            name=f"t0_{s}")
            t0s[s] = t0
            xv = xvs[s % nxp]
            for q in range(nchunks):
                r0 = (cfg.P // nchunks) * q    # pooled row of chunk start
                nr = cfg.P // nchunks          # pooled rows per chunk
                sc = scr[:, 0 : nr * cfg.W].rearrange("p (r c) -> p r c", c=cfg.W)
                nc.vector.tensor_add(
                    sc,
                    xv[:, 1 + 2 * r0 : 1 + 2 * (r0 + nr) : 2, 2 : 2 + cfg.W],
                    xv[:, 2 + 2 * r0 : 2 + 2 * (r0 + nr) : 2, 2 : 2 + cfg.W],
                )
                t0c = t0[:, r0 * cfg.PW : (r0 + nr) * cfg.PW].rearrange(
                    "p (r c) -> p r c", c=cfg.PW
                )
                nc.vector.tensor_add(
                    t0c, sc[:, :, 0 : cfg.W : 2], sc[:, :, 1 : cfg.W : 2]
                )

        def emit_conv1(s):
            t0v = t0s.pop(s)[:].rearrange("p (r c) -> p r c", c=cfg.PW)
            w1s = w1ss.pop(s)[:]
            for p in range(len(cfg.blk1) // 2):
                (yA, nA), (yB, nB) = cfg.blk1[2 * p], cfg.blk1[2 * p + 1]
                ps = tpsum.tile([128, nA * cfg.c1w], F32, tag="tps")
                for ky in range(3):
                    for kx in range(3):
                        t = ky * 3 + kx
                        lhs = w1s[:, t * 64 : (t + 1) * 64]
                        st, sp = t == 0, t == 8
                        nc.tensor.matmul(
                            ps[0:64, 0 : nA * cfg.c1w], lhs,
                            t0v[:, yA + ky : yA + ky + nA, kx : kx + cfg.c1w],
                            start=st, stop=sp,
                        )
                        nc.tensor.matmul(
                            ps[64:128, 0 : nB * cfg.c1w], lhs,
                            t0v[:, yB + ky : yB + ky + nB, kx : kx + cfg.c1w],
                            start=st, stop=sp,
                        )
                nc.scalar.activation(t1v[:, yA : yA + nA, :],
                                     ps[0:64, 0 : nA * cfg.c1w],
                                     AF.Lrelu, bias=tr_b[:, 0:1], alpha=0.01)
                nc.scalar.activation(t1v[:, yB : yB + nB, :],
                                     ps[64:128, 0 : nB * cfg.c1w],
                                     AF.Lrelu, bias=tr_b[:, 0:1], alpha=0.01)

        def emit_conv23(s):
            # conv2: two row blocks col-tiled
            (yA, nA), (yB, nB) = cfg.blk2
            ps = tpsum.tile([128, nA * cfg.c2w], F32, tag="tps")
            for ky in range(3):
                for kx in range(3):
                    t = ky * 3 + kx
                    lhs = w2T[:, t * 64 : (t + 1) * 64]
                    st, sp = t == 0, t == 8
                    nc.tensor.matmul(
                        ps[0:64, 0 : nA * cfg.c2w], lhs,
                        t1v[:, 2 * yA + ky : 2 * yA + ky + 2 * nA : 2,
                            kx : kx + 2 * cfg.c2w - 1 : 2],
                        start=st, stop=sp,
                    )
                    nc.tensor.matmul(
                        ps[64:128, 0 : nB * cfg.c2w], lhs,
                        t1v[:, 2 * yB + ky : 2 * yB + ky + 2 * nB : 2,
                            kx : kx + 2 * cfg.c2w - 1 : 2],
                        start=st, stop=sp,
                    )
            nc.scalar.activation(t2v[:, yA : yA + nA, :],
                                 ps[0:64, 0 : nA * cfg.c2w],
                                 AF.Lrelu, bias=tr_b[:, 1:2], alpha=0.01)
            nc.scalar.activation(t2v[:, yB : yB + nB, :],
                                 ps[64:128, 0 : nB * cfg.c2w],
                                 AF.Lrelu, bias=tr_b[:, 1:2], alpha=0.01)

            # conv3: two row blocks col-tiled
            (yA, nA), (yB, nB) = cfg.blk3
            ps = tpsum.tile([128, nA * cfg.c3w], F32, tag="tps")
            for ky in range(3):
                for kx in range(3):
                    t = ky * 3 + kx
                    lhs = w3T[:, t * 64 : (t + 1) * 64]
                    st, sp = t == 0, t == 8
                    nc.tensor.matmul(
                        ps[0:64, 0 : nA * cfg.c3w], lhs,
                        t2v[:, 2 * yA + ky : 2 * yA + ky + 2 * nA : 2,
                            kx : kx + 2 * cfg.c3w - 1 : 2],
                        start=st, stop=sp,
                    )
                    nc.tensor.matmul(
                        ps[64:128, 0 : nB * cfg.c3w], lhs,
                        t2v[:, 2 * yB + ky : 2 * yB + ky + 2 * nB : 2,
                            kx : kx + 2 * cfg.c3w - 1 : 2],
                        start=st, stop=sp,
                    )
            nc.scalar.activation(t3v[0:64, yA : yA + nA, :],
                                 ps[0:64, 0 : nA * cfg.c3w],
                                 AF.Lrelu, bias=tr_b[:, 2:3], alpha=0.01)
            nc.scalar.activation(t3v[0:64, yB : yB + nB, :],
                                 ps[64:128, 0 : nB * cfg.c3w],
                                 AF.Lrelu, bias=tr_b[:, 2:3], alpha=0.01)
            # duplicate t3 shifted by one col onto partitions 64-127
            # (tap-stacking source for the heads); on ScalarE so it sits
            # between the conv3 evacuation and the head ACTs
            nc.scalar.activation(
                t3v[64:128, :, 0 : cfg.c3w - 1], t3v[0:64, :, 1 : cfg.c3w],
                AF.Copy,
            )

        def emit_heads_att_synth(s):
            sm = spool.tile([128, 64], F32, tag="sm", name=f"sm_{s}")
            a_sb = sm[:, 0:1]
            att_bc = sm[:, 8:12]
            cc = sm[:, 12:16]
            gap = sm[:, 16:24]
            hscr = sm[:, 24:42].bitcast(BF16)[:, 0 : cfg.gapn]
            nh, nw = cfg.hh, cfg.hw
            # head 0 is the attention head (gap col 4); heads 1-4 -> cols 0-3
            gcol = [4, 0, 1, 2, 3]
            for h in range(5):
                ps = hpsum.tile([128, cfg.gapn], F32, tag="hps")
                for ky in range(3):
                    for g in range(2):
                        nc.tensor.matmul(
                            ps[64 * g : 64 * g + 64, :], hd_stk(h, ky, g),
                            t3v[:, ky : ky + 2 * nh - 1 : 2, 0 : 2 * nw - 1 : 2],
                            start=(ky == 0), stop=False,
                        )
                for ky in range(3):
                    for g in range(2):
                        nc.tensor.matmul(
                            ps[64 * g : 64 * g + 64, :], hd_sgl(h, ky, g),
                            t3v[0:64, ky : ky + 2 * nh - 1 : 2,
                                2 : 2 + 2 * nw - 1 : 2],
                            start=False, stop=(ky == 2),
                        )
                gc = gcol[h]
                nc.scalar.activation(
                    hscr, ps[:], AF.Identity, bias=head_b[:, h : h + 1],
                    accum_out=gap[:, gc : gc + 1],
                )
                if h == 0:
                    # attention chain, overlapping the remaining head matmuls:
                    # a = lrelu(gap4/gapn); att2row0 = a @ (att_fw.T/gapn);
                    # broadcast att+fb to all partitions via 2-row matmul
                    nc.scalar.activation(a_sb, gap[:, 4:5], AF.Lrelu,
                                         scale=1.0 / cfg.gapn, alpha=0.01)
                    ps_a = hpsum.tile([1, 4], F32, tag="hps")
                    nc.tensor.matmul(ps_a[:], a_sb, att_fwT, start=True,
                                     stop=True)
                    nc.scalar.activation(att2[0:1, :], ps_a[:], AF.Copy)
                    ps_b = hpsum.tile([128, 4], F32, tag="hps")
                    nc.tensor.matmul(ps_b[:], ones2, att2, start=True,
                                     stop=True)
                    nc.scalar.activation(att_bc, ps_b[:], AF.Copy)
            nc.vector.tensor_mul(cc, att_bc, gap[:, 0:4])
            # fold wm_coff into the per-expert coefficients
            nc.vector.tensor_scalar_mul(cc, cc, wmc[:, s : s + 1])

            # synthesize w_dynT[i, (kh kw o)]; tree-shaped so the DVE ops
            # have no serial chain to stretch
            A = ypool.tile([128, 9 * 64], BF16, tag="synA", name=f"synA_{s}")
            Bt = ypool.tile([128, 9 * 64], BF16, tag="synB", name=f"synB_{s}")
            wdyn = wpool.tile([128, 9 * 64], BF16, tag="wdyn", name=f"wdyn_{s}")
            wdyns[s] = wdyn
            nc.vector.tensor_scalar_mul(A[:], expT[:, 0:576], cc[:, 0:1])
            nc.vector.tensor_scalar_mul(Bt[:], expT[:, 1152:1728], cc[:, 2:3])
            nc.vector.scalar_tensor_tensor(
                A[:], expT[:, 576:1152], cc[:, 1:2], A[:], op0=OP.mult,
                op1=OP.add,
            )
            nc.vector.scalar_tensor_tensor(
                Bt[:], expT[:, 1728:2304], cc[:, 3:4], Bt[:], op0=OP.mult,
                op1=OP.add,
            )
            nc.vector.tensor_add(wdyn[:], A[:], Bt[:])

        def emit_main_group(s, q):
            xv = xvs[s % nxp]
            wdyn = wdyns[s]
            out_t = opool.tile([128, cfg.POUT * 512], BF16, tag="outsb",
                               name=f"out_{s}_{q}")
            for j in range(cfg.POUT):
                pair = q * cfg.POUT + j
                # even/odd row-groups stream concurrently into the two
                # PE column groups
                ps = mpsum.tile([128, 512], F32, tag="mps")
                for ky in range(3):
                    for kx in range(3):
                        for half in range(2):
                            y0 = (2 * pair + half) * cfg.RPG
                            nc.tensor.matmul(
                                ps[half * 64 : half * 64 + 64, :],
                                wdyn[:, (ky * 3 + kx) * 64 : (ky * 3 + kx + 1) * 64],
                                xv[:, y0 + ky : y0 + ky + cfg.RPG,
                                   1 + kx : 1 + kx + cfg.W],
                                start=(ky == 0 and kx == 0),
                                stop=(ky == 2 and kx == 2),
                            )
                nc.scalar.activation(
                    out_t[:, j * 512 : (j + 1) * 512], ps[:], AF.Copy
                )
                # last sample: stream the output out per psum-pair so the
                # final DMA tail is short
                if s == cfg.BL - 1 and j % 2 == 1:
                    nc.gpsimd.dma_start(
                        y[s, q, :, (j - 1) * 512 : (j + 1) * 512],
                        out_t[:, (j - 1) * 512 : (j + 1) * 512],
                    )
            if s != cfg.BL - 1:
                nc.gpsimd.dma_start(y[s, q], out_t[:])
            if q == cfg.NOUT - 1:
                wdyns.pop(s)

        # ---------- software pipeline ----------
        emit_dma(0, nchunks=8)
        nc.gpsimd.dma_start(bhd[:], bhd_d[:])
        nc.gpsimd.dma_start(bex[:], bex_d[:])
        # PE warm-up: dummy matmuls on a memset tile while the image DMA
        # lands, so the HAM clock gate is released before real work starts
        wdum = cpool.tile([128, 576], BF16, tag="wdum")
        nc.vector.memset(wdum[:], 0.0)
        wps = mpsum.tile([128, 512], F32, tag="mps", name="warmup")
        for i in range(28):
            nc.tensor.matmul(wps[0:64, 0:512], wdum[:, 0:64],
                             wdum[:, 64:576], start=(i == 0), stop=(i == 27))
        emit_w1s(0)
        emit_pool(0, nchunks=8)
        if cfg.BL > 1:
            emit_dma(1)
        emit_conv1(0)
        emit_conv23(0)
        emit_heads_att_synth(0)
        if cfg.BL > 1:
            emit_w1s(1)
            emit_pool(1)

        def stage_after(s, q):
            if q == min(0, cfg.NOUT - 1):
                if s + 2 < cfg.BL:
                    emit_dma(s + 2)
                if s + 1 < cfg.BL:
                    emit_conv1(s + 1)
            if q == min(1, cfg.NOUT - 1):
                if s + 1 < cfg.BL:
                    emit_conv23(s + 1)
                    emit_heads_att_synth(s + 1)
            if q == min(2, cfg.NOUT - 1):
                if s + 2 < cfg.BL:
                    emit_w1s(s + 2)
                    emit_pool(s + 2)

        for s in range(cfg.BL):
            for q in range(cfg.NOUT):
                emit_main_group(s, q)
                stage_after(s, q)

    return nc


_NC_CACHE = {}
TRACE = False       # set by test harness to collect an NTFF profile
TRACE_DIR = None    # where to leave the NTFF/perfetto artifacts
LAST_RESULT = None  # BassKernelResults of the most recent kernel() call


def _get_nc(cfg):
    key = (cfg.BL, cfg.Cin, cfg.H, cfg.W)
    if key not in _NC_CACHE:
        nc = build_nc(cfg)
        if not nc.is_finalized():
            nc.finalize()
        _NC_CACHE[key] = nc
    return _NC_CACHE[key]


def pad_images(cfg, x):
    """[n, Cin, H, W] -> zero-padded flat bf16 [n, Cin, Hp*Wp]."""
    n = x.shape[0]
    xp = np.zeros((n, cfg.Cin, cfg.Hp, cfg.Wp), ml_dtypes.bfloat16)
    xp[:, :, 1 : cfg.H + 1, 2 : cfg.W + 2] = x.astype(ml_dtypes.bfloat16)
    return xp.reshape(n, cfg.Cin, cfg.NPAD)


def unpack_y(cfg, yraw):
    """[BL, NOUT, 128, POUT*512] bf16 -> [BL, 64, H, W] f32."""
    a = np.asarray(yraw).astype(np.float32)
    a = a.reshape(cfg.BL, cfg.NOUT, 2, 64, cfg.POUT, cfg.RPG, cfg.W)
    a = a.transpose(0, 3, 1, 4, 2, 5, 6)
    return np.ascontiguousarray(a.reshape(cfg.BL, 64, cfg.H, cfg.W))


def kernel(**inputs):
    x = np.asarray(inputs["x"], np.float32)
    B, Cin, H, W = x.shape
    cfg = Cfg(BL=B // 8, Cin=Cin, H=H, W=W)
    nc = _get_nc(cfg)
    wnames = [
        "wm_w1", "wm_b1", "wm_w2", "wm_b2", "tr_w1", "tr_b1", "tr_w2", "tr_b2",
        "tr_w3", "tr_b3", "t1_w", "t1_b", "t2_w", "t2_b", "t3_w", "t3_b",
        "t4_w", "t4_b", "att_cw", "att_cb", "att_fw", "att_fb", "expert_w",
    ]
    ws = {k: np.asarray(inputs[k], np.float32) for k in wnames}
    wm = np.asarray(inputs["wm"], np.float32)
    in_maps = []
    for c in range(8):
        sl = slice(c * cfg.BL, (c + 1) * cfg.BL)
        bwm, btw, bhd, bex = make_blobs(cfg, wm[sl], **ws)
        in_maps.append({"x": pad_images(cfg, x[sl]), "bwm": bwm, "btw": btw,
                        "bhd": bhd, "bex": bex})
    global LAST_RESULT
    kw = {"tmpdir": TRACE_DIR} if (TRACE and TRACE_DIR) else {}
    res = run_bass_kernel_spmd(nc, in_maps, list(range(8)), trace=TRACE, **kw)
    LAST_RESULT = res
    return np.concatenate(
        [unpack_y(cfg, res.results[c]["y"]) for c in range(8)], axis=0
    )
